# revision 1
# baseline (speedup 1.0000x reference)
"""GNN message passing (gather + segment_sum) on 8 Trainium2 NeuronCores.

Strategy: node-shard the 100000 target nodes across the 8 cores (12500
each), and route every edge to the core that owns its target node — no
cross-core reduction needed.  Per core, edges are bucketed by 128-node
target windows and by 32768-row source chunks (dma_gather indices are
int16).  The device kernel, per batch of windows:

  1. bulk-gathers source rows X2[j] (256-byte padded rows) with one
     `dma_gather` per source chunk into SBUF ([128, slots, 64] layout:
     edge i of the instruction lands on partition i%128, slot i//128),
  2. builds a one-hot selection matrix S[e, m] = (local_target[e] == m)
     with a broadcast DVE `is_equal` per (window, chunk) group,
  3. segment-sums on the tensor engine: psum[m, :] += S^T @ gathered,
     accumulating all of a window's tiles into one PSUM tile,
  4. copies each finished 128-node window into an SBUF output buffer,
     written out with two final DMAs.

Per-(window, chunk) group sizes are shared across cores (max over cores,
rounded to 128), so all 8 cores run one SPMD program; slack slots gather
chunk row 0 with local index -1 (one-hot row of zeros).
"""

import numpy as np

N_NODES = 100000
N_EDGES = 1600000
D = 32             # feature dim
DP = 64            # padded row (256 B)
C = 8              # cores
P = 128            # partitions / edges per tile
W = 128            # target-node window
CH = 32768         # source chunk rows (int16 index range)
B_WIN = 8          # windows per gather batch


def _shapes():
    npc = N_NODES // C
    nwin = (npc + W - 1) // W
    lastw = npc - (nwin - 1) * W
    nch = (N_NODES + CH - 1) // CH
    return npc, nwin, lastw, nch


def _prep(edge_index):
    """Bucket/pad edges by (batch, chunk, window); build device arrays."""
    npc, nwin, _, nch = _shapes()
    ei = np.asarray(edge_index)
    tgt = ei[:, 0].astype(np.int64)
    src = ei[:, 1].astype(np.int64)
    n_edges = tgt.shape[0]

    core = tgt // npc
    ilocal = tgt - core * npc
    w_arr = ilocal // W
    li = (ilocal - w_arr * W).astype(np.float32)
    c_arr = src // CH
    b_arr = w_arr // B_WIN

    # counts per (core, window, chunk)
    key = (core * nwin + w_arr) * nch + c_arr
    counts = np.bincount(key, minlength=C * nwin * nch).reshape(C, nwin, nch)
    grp_tiles = -(-counts.max(axis=0) // P)          # [nwin, nch]
    # guarantee at least one tile per window so PSUM is always written
    empty = grp_tiles.sum(axis=1) == 0
    grp_tiles[empty, 0] = 1

    # group ordering: batch -> chunk -> window
    nbatch = (nwin + B_WIN - 1) // B_WIN
    gs_tiles = np.zeros((nwin, nch), np.int64)       # global start tile
    k = 0
    batches = []   # per batch: (tile0, n_tiles, per-chunk (t0, tn), windows)
    for b in range(nbatch):
        wins = list(range(b * B_WIN, min((b + 1) * B_WIN, nwin)))
        tile0 = k
        per_chunk = []
        for c in range(nch):
            t0 = k
            for w in wins:
                gs_tiles[w, c] = k
                k += int(grp_tiles[w, c])
            per_chunk.append((t0, k - t0))
        batches.append((tile0, k - tile0, per_chunk, wins))
    k_tot = k

    # place edges: sort by (core, b, c, w); compute position within group
    order = np.lexsort((src, w_arr, c_arr, b_arr, core))
    # rank groups in the same (core, b, c, w) order
    g_core = np.repeat(np.arange(C), nwin * nch)
    g_w = np.tile(np.repeat(np.arange(nwin), nch), C)
    g_c = np.tile(np.arange(nch), C * nwin)
    g_b = g_w // B_WIN
    g_order = np.lexsort((g_w, g_c, g_b, g_core))
    g_rank = np.empty(C * nwin * nch, np.int64)
    g_rank[g_order] = np.arange(C * nwin * nch)
    counts_flat = counts.reshape(-1)
    counts_sorted = counts_flat[g_order]
    gstarts_sorted = np.zeros(C * nwin * nch, np.int64)
    gstarts_sorted[1:] = np.cumsum(counts_sorted)[:-1]

    key_s = key[order]
    pos = np.arange(n_edges, dtype=np.int64) - gstarts_sorted[g_rank[key_s]]
    slot = gs_tiles[w_arr[order], c_arr[order]] * P + pos

    offs_all = np.zeros((C, k_tot * P), np.int16)
    li_all = np.full((C, k_tot * P), -1.0, np.float32)
    core_s = core[order]
    offs_all[core_s, slot] = (src[order] - c_arr[order] * CH).astype(np.int16)
    li_all[core_s, slot] = li[order]

    # li device layout: edge slot e -> (partition e % P, col e // P)
    li_dev = np.ascontiguousarray(
        li_all.reshape(C, k_tot, P).transpose(0, 2, 1))          # [C, P, K]
    # idx device layout: per instruction (b, c), flat idx list wrapped in 16
    # partitions and replicated 8x; concatenated along the free dim.
    offs_cols = []
    for (tile0, n_tiles, per_chunk, wins) in batches:
        for (t0, tn) in per_chunk:
            if tn == 0:
                continue
            flat = offs_all[:, t0 * P:(t0 + tn) * P]             # [C, tn*128]
            wrapped = flat.reshape(C, tn * 8, 16).transpose(0, 2, 1)
            offs_cols.append(np.tile(wrapped, (1, 8, 1)))        # [C,128,tn*8]
    offs_dev = np.ascontiguousarray(np.concatenate(offs_cols, axis=2))
    return offs_dev, li_dev, grp_tiles, gs_tiles, batches, k_tot


def _emit(nc, bass, mybir, tile, mlp, grp_tiles, gs_tiles, batches, k_tot):
    """Declare IO tensors and build the SPMD program on `nc`."""
    npc, nwin, lastw, nch = _shapes()
    x2_d = nc.dram_tensor("x2", [N_NODES, DP], mybir.dt.float32,
                          kind="ExternalInput")
    offs_d = nc.dram_tensor("offs", [P, k_tot * 8], mybir.dt.int16,
                            kind="ExternalInput")
    li_d = nc.dram_tensor("li", [P, k_tot], mybir.dt.float32,
                          kind="ExternalInput")
    iota_d = nc.dram_tensor("iota", [P, W], mybir.dt.float32,
                            kind="ExternalInput")
    out_d = nc.dram_tensor("out", [npc, D], mybir.dt.float32,
                           kind="ExternalOutput")

    max_bt = max(nt for (_, nt, _, _) in batches)      # tiles per batch
    max_gt = int(grp_tiles.max())                      # tiles per group

    with tile.TileContext(nc) as tc:
        with (
            tc.tile_pool(name="const", bufs=1) as cpool,
            tc.tile_pool(name="sel", bufs=3) as spool,
            tc.tile_pool(name="ps", bufs=4, space="PSUM") as ppool,
        ):
            offs_t = cpool.tile([P, k_tot * 8], mybir.dt.int16)
            li_t = cpool.tile([P, k_tot], mybir.dt.float32)
            iota_t = cpool.tile([P, W], mybir.dt.float32)
            o_t = cpool.tile([P, nwin * D], mybir.dt.float32)
            # explicit ping-pong gather buffers: batch b uses buffer b % 2,
            # so padding-slot skipping (idx -1) sees batch b-2's finite data
            g_bufs = [cpool.tile([P, max_bt * DP], mybir.dt.float32,
                                 name=f"gbuf{i}", tag=f"gbuf{i}")
                      for i in range(2)]

            nc.gpsimd.load_library(mlp)
            nc.sync.dma_start(out=iota_t[:], in_=iota_d[:])

            icol = 0   # running idx column (in tiles) within offs_t
            for bi, (tile0, n_tiles, per_chunk, wins) in enumerate(batches):
                g_t = g_bufs[bi % 2]
                nc.sync.dma_start(
                    out=offs_t[:, icol * 8:(icol + n_tiles) * 8],
                    in_=offs_d[:, icol * 8:(icol + n_tiles) * 8])
                nc.sync.dma_start(
                    out=li_t[:, tile0:tile0 + n_tiles],
                    in_=li_d[:, tile0:tile0 + n_tiles])
                for c, (t0, tn) in enumerate(per_chunk):
                    if tn == 0:
                        continue
                    rows = min(CH, N_NODES - c * CH)
                    r0 = t0 - tile0
                    nc.gpsimd.dma_gather(
                        out_ap=g_t[:, r0 * DP:(r0 + tn) * DP].rearrange(
                            "p (k d) -> p k d", d=DP),
                        in_ap=x2_d[c * CH:c * CH + rows, :],
                        idxs_ap=offs_t[:, icol * 8:(icol + tn) * 8],
                        num_idxs=tn * P,
                        num_idxs_reg=tn * P,
                        elem_size=DP,
                        single_packet=False,
                    )
                    icol += tn
                for w in wins:
                    ps = ppool.tile([P, D], mybir.dt.float32)
                    w_tiles = []
                    for c in range(nch):
                        gt = int(grp_tiles[w, c])
                        if gt:
                            w_tiles.append((int(gs_tiles[w, c]), gt))
                    n_w = sum(gt for _, gt in w_tiles)
                    done = 0
                    for (t0, gt) in w_tiles:
                        s_t = spool.tile([P, max_gt * W], mybir.dt.float32,
                                         tag="s")
                        nc.vector.tensor_tensor(
                            out=s_t[:, :gt * W].rearrange(
                                "p (k m) -> p k m", k=gt),
                            in0=li_t[:, t0:t0 + gt].to_broadcast([P, gt, W]),
                            in1=iota_t[:].rearrange(
                                "p (o m) -> p o m", o=1).to_broadcast(
                                    [P, gt, W]),
                            op=mybir.AluOpType.is_equal,
                        )
                        for t in range(gt):
                            rel = t0 - tile0 + t
                            nc.tensor.matmul(
                                out=ps[:],
                                lhsT=s_t[:, t * W:(t + 1) * W],
                                rhs=g_t[:, rel * DP:rel * DP + D],
                                start=(done == 0),
                                stop=(done == n_w - 1),
                            )
                            done += 1
                    nc.vector.tensor_copy(
                        out=o_t[:, w * D:(w + 1) * D], in_=ps[:])

                w0, w1 = wins[0], wins[-1]
                wf = min(w1, nwin - 2)          # last full window this batch
                if wf >= w0:
                    nc.sync.dma_start(
                        out=out_d[w0 * W:(wf + 1) * W].rearrange(
                            "(w p) f -> p w f", p=P),
                        in_=o_t[:].rearrange(
                            "p (w f) -> p w f", w=nwin)[:, w0:wf + 1, :],
                    )
                if w1 == nwin - 1:
                    nc.sync.dma_start(
                        out=out_d[(nwin - 1) * W:],
                        in_=o_t[:lastw, (nwin - 1) * D:nwin * D],
                    )


def kernel(X, edge_index, **run_kwargs):
    import sys
    if "/opt/trn_rl_repo" not in sys.path:
        sys.path.insert(0, "/opt/trn_rl_repo")
    import concourse.bass as bass
    import concourse.bacc as bacc
    import concourse.mybir as mybir
    from concourse import tile
    from concourse.library_config import mlp
    from concourse.bass_utils import run_bass_kernel_spmd

    X = np.asarray(X)
    x2 = np.zeros((N_NODES, DP), np.float32)
    x2[:, :D] = X
    offs_dev, li_dev, grp_tiles, gs_tiles, batches, k_tot = _prep(edge_index)
    iota_host = np.ascontiguousarray(
        np.broadcast_to(np.arange(W, dtype=np.float32), (P, W)))

    nc = bacc.Bacc("TRN2", target_bir_lowering=False, debug=False,
                   num_devices=C)
    _emit(nc, bass, mybir, tile, mlp, grp_tiles, gs_tiles, batches, k_tot)
    nc.compile()

    in_maps = [
        {"x2": x2, "offs": offs_dev[c], "li": li_dev[c], "iota": iota_host}
        for c in range(C)
    ]
    res = run_bass_kernel_spmd(nc, in_maps, list(range(C)), **run_kwargs)
    out = np.concatenate([res.results[c]["out"] for c in range(C)], axis=0)
    kernel.last_nc = nc
    kernel.last_results = res
    return out

